# revision 12
# baseline (speedup 1.0000x reference)
"""Distributed Trainium2 kernel for nn_Attention_2654289789382 (sparse_attention).

Math (reference):
    sigma = sigmoid(x @ W_sigma + b_sigma)           (b, h, n)
    den_i = exp(sigma)+1 ;  r_i = 1/den_i = sigmoid(-sigma)   in (0.2689, 0.5)
    prior[i,j] = softmax_j(-|i-j| * r_i)
    out = (prior @ v) reshaped @ W_out + b_out,  v = x @ W_v

Structure exploited:
  * r_i >= 0.2689  =>  banded attention, half-width 64: per 128-row i-block
    only 2 j-tiles of 128 (at +-64 offsets) contribute. x context trimmed
    to exactly the used 1152 rows per core.
  * softmax denominator in closed form (two-sided geometric series).
  * No Sigmoid table: sigma and r = 1/den via Exp + DVE reciprocal only;
    Exp table preloaded at t=0 by a dummy activation.
  * HAM warmup matmuls at t=0 so real GEMMs run at 2.4 GHz.
  * Loads striped over all 3 DMA issuers (sync/scalar HWDGE + gpsimd
    SWDGE, ~100 GB/s each), sigma-critical pieces first.
  * Tensor-queue order: warmup, sigma GEMM, V GEMM, then the tiny
    transposes (they wait on the sigma chain - placing them before V
    head-of-line-blocks the PE for ~7us).
  * -r / 1/den staged to DRAM h-major; R_all broadcast = per-head
    partition-striped stride-0 DMAs across all 3 issuers, interleaved
    with Iv pieces in consumption order.
  * Q = exp(|i-j| * -r): one bf16 DVE mult per (ch,hp,hh), ONE
    [128,2048] ScalarE Exp per (ch,hp). AV bf16, 2 heads / psum tile,
    4 blocks share a [128,512] psum so normalization is one DVE op per
    (ch,hp). out^T lands in proj lhsT layout; proj+bias+store per block.
  * Output stored bf16 (halves store traffic; ~1e-3 extra rel err).

Sharding: 8 cores = 4 batches x 2 sequence halves; no collectives.
"""

import numpy as np
import ml_dtypes

import concourse.bass as bass
import concourse.mybir as mybir
import concourse.tile as tile
from concourse import bacc
from concourse.bass_utils import run_bass_kernel_spmd

F32 = mybir.dt.float32
BF16 = mybir.dt.bfloat16

B, N, D = 4, 2048, 512
H, DH = 8, 64
HALF = N // 2            # 1024 rows per core
W = 64                   # band half-width
NJ = HALF + 2 * W        # 1152 used j rows per core
NBLK = HALF // 128       # 8 i-blocks per core
NVT = 9                  # V tiles: rows 64k..64k+128 within the 1152
CB = 4                   # i-blocks per chunk
NCH = NBLK // CB         # chunks

_nc_cache = None


def _build_nc():
    nc = bacc.Bacc("TRN2", target_bir_lowering=False, debug=False)

    xt0 = nc.dram_tensor("xt0", [128, NJ], BF16, kind="ExternalInput")
    xt1 = nc.dram_tensor("xt1", [128, NJ], BF16, kind="ExternalInput")
    xt2 = nc.dram_tensor("xt2", [128, NJ], BF16, kind="ExternalInput")
    xt3 = nc.dram_tensor("xt3", [128, NJ], BF16, kind="ExternalInput")
    wvb4 = nc.dram_tensor("wvb4", [128, 4 * D], BF16, kind="ExternalInput")
    wob4 = nc.dram_tensor("wob4", [128, 4 * D], BF16, kind="ExternalInput")
    wsb4 = nc.dram_tensor("wsb4", [128, 4 * H], BF16, kind="ExternalInput")
    cpak_bf = nc.dram_tensor("cpak_bf", [128, 384], BF16, kind="ExternalInput")
    cpak_f32 = nc.dram_tensor("cpak_f32", [128, 136], F32, kind="ExternalInput")
    bout = nc.dram_tensor("bout", [128, D], F32, kind="ExternalInput")
    out = nc.dram_tensor("out", [HALF, D], BF16, kind="ExternalOutput")
    negr_d = nc.dram_tensor("negr_d", [64, 128], BF16)
    inv_d = nc.dram_tensor("inv_d", [64, 128], BF16)

    EXP = mybir.ActivationFunctionType.Exp
    MUL = mybir.AluOpType.mult
    ADD = mybir.AluOpType.add

    with tile.TileContext(nc) as tc:
        with (
            tc.tile_pool(name="const", bufs=1) as cpool,
            tc.tile_pool(name="vpool", bufs=1) as vpool,
            tc.tile_pool(name="otpool", bufs=1) as otpool,
            tc.tile_pool(name="sg", bufs=1) as sgpool,
            tc.tile_pool(name="bc", bufs=1) as bcpool,
            tc.tile_pool(name="warm", bufs=1) as wpool,
        ):
            # ---- t=0: exp-table preload + HAM warmup (no data deps) ----
            dum = wpool.tile([128, 128], BF16, tag="dum")
            nc.vector.memset(dum[:], 0.25)
            dume = wpool.tile([128, 8], F32, tag="dume")
            nc.scalar.activation(dume[:], dum[:, 0:8], EXP)
            with tc.tile_pool(name="pswarm", bufs=1, space="PSUM") as psw:
                pw = psw.tile([128, 128], F32, tag="pw")
                for _ in range(40):
                    nc.tensor.matmul(pw[:], lhsT=dum[:], rhs=dum[:],
                                     start=True, stop=True)

            # ------------- loads: 3 issuers, critical-first -------------
            cf32_t = cpool.tile([128, 136], F32, tag="cf32")
            nc.sync.dma_start(cf32_t[:], cpak_f32[:, :])
            cbf_t = cpool.tile([128, 384], BF16, tag="cbf")
            nc.scalar.dma_start(cbf_t[:], cpak_bf[:, :])
            wsb_t = cpool.tile([128, 4 * H], BF16, tag="wsb")
            nc.gpsimd.dma_start(wsb_t[:], wsb4[:, :])

            xt_t = []
            for d, (dram, eng) in enumerate(
                [(xt0, nc.sync), (xt1, nc.scalar), (xt2, nc.gpsimd)]
            ):
                t = cpool.tile([128, NJ], BF16, tag=f"xt{d}")
                eng.dma_start(t[:], dram[:, :])
                xt_t.append(t)
            t3 = cpool.tile([128, NJ], BF16, tag="xt3")
            nc.sync.dma_start(t3[:, 0:576], xt3[:, 0:576])
            nc.scalar.dma_start(t3[:, 576:NJ], xt3[:, 576:NJ])
            xt_t.append(t3)

            wvb_t = cpool.tile([128, 4 * D], BF16, tag="wvb")
            nc.sync.dma_start(wvb_t[:, 0:D], wvb4[:, 0:D])
            nc.scalar.dma_start(wvb_t[:, D:2 * D], wvb4[:, D:2 * D])
            nc.gpsimd.dma_start(wvb_t[:, 2 * D:4 * D], wvb4[:, 2 * D:4 * D])
            wob_t = cpool.tile([128, 4 * D], BF16, tag="wob")
            bout_t = cpool.tile([128, D], F32, tag="bout")

            m2r_t = cbf_t[:, 0:256]
            identb = cbf_t[:, 256:384]
            bsig = cf32_t[:, 0:8]
            ivp1 = cf32_t[:, 8:72]
            ivnm = cf32_t[:, 72:136]

            # ------------- sigma GEMM (blocks at cols 64 + 128b) --------
            with tc.tile_pool(name="pss", bufs=1, space="PSUM") as pss:
                ps = pss.tile([128, NBLK * H], F32, tag="ps")
                for b in range(NBLK):
                    for dt in range(4):
                        nc.tensor.matmul(
                            ps[:, b * H:(b + 1) * H],
                            lhsT=xt_t[dt][:, 64 + b * 128:64 + (b + 1) * 128],
                            rhs=wsb_t[:, dt * H:(dt + 1) * H],
                            start=(dt == 0),
                            stop=(dt == 3),
                        )
                s_all = sgpool.tile([128, NBLK * H], F32, tag="s_all")
                nc.vector.tensor_tensor(
                    s_all[:].rearrange("p (b h) -> p b h", h=H),
                    ps[:].rearrange("p (b h) -> p b h", h=H),
                    bsig.rearrange("p (one h) -> p one h", one=1)
                    .broadcast_to((128, NBLK, H)),
                    op=ADD,
                )

            # ---------------- V = x @ W_v (9 tiles) ---------------------
            V_t = []
            with (
                tc.tile_pool(name="psv", bufs=3, space="PSUM") as psv,
                tc.tile_pool(name="pst", bufs=1, space="PSUM") as pst,
            ):
                for k in range(NVT):
                    pv = psv.tile([128, D], F32, tag="pv")
                    for dt in range(4):
                        nc.tensor.matmul(
                            pv[:],
                            lhsT=xt_t[dt][:, 128 * k:128 * k + 128],
                            rhs=wvb_t[:, dt * D:(dt + 1) * D],
                            start=(dt == 0),
                            stop=(dt == 3),
                        )
                    vt = vpool.tile([128, D], BF16, tag=f"V{k}")
                    if k % 3 == 2:
                        nc.scalar.copy(vt[:], pv[:])
                    else:
                        nc.vector.tensor_copy(vt[:], pv[:])
                    V_t.append(vt)

                # ---- sigma chain (scalar/DVE; PE only for transposes) --
                ems = sgpool.tile([128, NBLK * H], F32, tag="ems")
                nc.scalar.activation(ems[:], s_all[:], EXP, scale=-1.0)
                d1 = sgpool.tile([128, NBLK * H], F32, tag="d1")
                nc.vector.tensor_scalar(d1[:], ems[:], 1.0, None, ADD)
                sig = sgpool.tile([128, NBLK * H], F32, tag="sig")
                nc.vector.reciprocal(sig[:], d1[:])
                esg = sgpool.tile([128, NBLK * H], F32, tag="esg")
                nc.scalar.activation(esg[:], sig[:], EXP)
                den = sgpool.tile([128, NBLK * H], F32, tag="den")
                nc.vector.tensor_scalar(den[:], esg[:], 1.0, None, ADD)
                r_all = sgpool.tile([128, NBLK * H], F32, tag="r_all")
                nc.vector.reciprocal(r_all[:], den[:])

                negr_b = sgpool.tile([128, NBLK * H], BF16, tag="negr_b")
                nc.vector.tensor_scalar(
                    negr_b[:].rearrange("p (h b) -> p h b", b=NBLK),
                    r_all[:].rearrange("p (b h) -> p h b", h=H),
                    -1.0, None, MUL,
                )
                ptn = pst.tile([64, 128], BF16, tag="ptn")
                nc.tensor.transpose(ptn[:], negr_b[:], identb)
                negrT = sgpool.tile([64, 128], BF16, tag="negrT")
                nc.scalar.copy(negrT[:], ptn[:])
                nc.sync.dma_start(negr_d.ap(), negrT[:, :])

                # ---- 1/den closed form ----
                z = sgpool.tile([128, NBLK * H], F32, tag="z")
                nc.scalar.activation(z[:], r_all[:], EXP, scale=-1.0)
                argA = sgpool.tile([128, NBLK * H], F32, tag="argA")
                nc.vector.tensor_mul(argA[:], r_all[:], ivp1)
                expA = sgpool.tile([128, NBLK * H], F32, tag="expA")
                nc.scalar.activation(expA[:], argA[:], EXP)
                argB = sgpool.tile([128, NBLK * H], F32, tag="argB")
                nc.vector.tensor_mul(argB[:], r_all[:], ivnm)
                expB = sgpool.tile([128, NBLK * H], F32, tag="expB")
                nc.scalar.activation(expB[:], argB[:], EXP)
                w = sgpool.tile([128, NBLK * H], F32, tag="w")
                nc.vector.tensor_scalar(w[:], z[:], -1.0, 1.0, MUL, ADD)
                t1 = sgpool.tile([128, NBLK * H], F32, tag="t1")
                nc.vector.tensor_scalar_mul(t1[:], z[:], 2.0)
                nc.vector.tensor_sub(t1[:], t1[:], expA[:])
                nc.vector.tensor_sub(t1[:], t1[:], expB[:])
                u = sgpool.tile([128, NBLK * H], F32, tag="u")
                nc.vector.tensor_add(u[:], w[:], t1[:])
                ru = sgpool.tile([128, NBLK * H], F32, tag="ru")
                nc.vector.reciprocal(ru[:], u[:])
                inv_c = sgpool.tile([128, NBLK * H], F32, tag="inv_c")
                nc.vector.tensor_mul(inv_c[:], w[:], ru[:])
                inv_b = sgpool.tile([128, NBLK * H], BF16, tag="inv_b")
                nc.vector.tensor_copy(
                    inv_b[:].rearrange("p (h b) -> p h b", b=NBLK),
                    inv_c[:].rearrange("p (b h) -> p h b", h=H),
                )
                pti = pst.tile([64, 128], BF16, tag="pti")
                nc.tensor.transpose(pti[:], inv_b[:], identb)
                invT = sgpool.tile([64, 128], BF16, tag="invT")
                nc.scalar.copy(invT[:], pti[:])
                nc.scalar.dma_start(inv_d.ap(), invT[:, :])

            # ---- broadcasts, split per (piece, chunk), in consumption
            #      order, round-robined across the 3 DMA issuers.  wob /
            #      bout inserted after the ch0 pieces (needed ~first proj).
            R_all = bcpool.tile([128, H * HALF], BF16, tag="R_all")
            Iv_pair = bcpool.tile([128, 4 * HALF], BF16, tag="Iv_pair")
            ISS = [nc.sync, nc.scalar, nc.gpsimd]
            nsrc = negr_d.ap().rearrange("r p -> (r p)").unsqueeze(0)
            isrc = inv_d.ap().rearrange("r p -> (r p)").unsqueeze(0)

            def jobs_for_chunk(ch):
                c0 = ch * 512
                jobs = []
                for hp in range(4):
                    for hh in range(2):
                        h = 2 * hp + hh
                        for p0, p1 in ((0, 64), (64, 128)):
                            jobs.append((
                                R_all[p0:p1, h * HALF + c0:h * HALF + c0 + 512],
                                nsrc[:, h * HALF + c0:h * HALF + c0 + 512]
                                .to_broadcast((64, 512)),
                            ))
                    for half in range(2):
                        h = 2 * hp + half
                        jobs.append((
                            Iv_pair[half * 64:(half + 1) * 64,
                                    hp * HALF + c0:hp * HALF + c0 + 512],
                            isrc[:, h * HALF + c0:h * HALF + c0 + 512]
                            .to_broadcast((64, 512)),
                        ))
                return jobs

            ji = 0
            for dst, src in jobs_for_chunk(0):
                ISS[ji % 3].dma_start(dst, src)
                ji += 1
            # wob / bout land between ch0 and ch1 broadcast waves
            nc.sync.dma_start(wob_t[:, 0:2 * D], wob4[:, 0:2 * D])
            nc.scalar.dma_start(wob_t[:, 2 * D:4 * D], wob4[:, 2 * D:4 * D])
            nc.gpsimd.dma_start(bout_t[:], bout[:, :])
            for dst, src in jobs_for_chunk(1):
                ISS[ji % 3].dma_start(dst, src)
                ji += 1

            outT_t = []
            for t in range(4):
                oT = otpool.tile([128, HALF], BF16, tag=f"oT{t}")
                outT_t.append(oT)

            # ---------------- main loop ----------------
            with (
                tc.tile_pool(name="qp", bufs=4) as qpool,
                tc.tile_pool(name="fin", bufs=3) as fpool,
                tc.tile_pool(name="psa", bufs=3, space="PSUM") as psa,
                tc.tile_pool(name="psf", bufs=2, space="PSUM") as psf,
            ):
                for ch in range(NCH):
                    for hp in range(4):
                        Q = qpool.tile([128, 2 * CB * 256], BF16, tag="Q")
                        ARG = qpool.tile([128, 2 * CB * 256], BF16, tag="ARG")
                        for hh in range(2):
                            h = 2 * hp + hh
                            R = R_all[:, h * HALF + ch * CB * 128:
                                      h * HALF + (ch + 1) * CB * 128]
                            nc.vector.tensor_tensor(
                                ARG[:, hh * 1024:(hh + 1) * 1024]
                                .rearrange("p (b o q) -> p b o q", b=CB, o=2),
                                m2r_t
                                .rearrange("p (one o q) -> p one o q", one=1, o=2)
                                .broadcast_to((128, CB, 2, 128)),
                                R.rearrange("p (b one q) -> p b one q", b=CB, one=1)
                                .broadcast_to((128, CB, 2, 128)),
                                op=MUL,
                            )
                        nc.scalar.activation(Q[:], ARG[:], EXP)
                        pav = psa.tile([128, CB * 128], F32, tag="pav")
                        for bi in range(CB):
                            b = ch * CB + bi
                            for hh in range(2):
                                h = 2 * hp + hh
                                for o in range(2):
                                    nc.tensor.matmul(
                                        pav[hh * 64:(hh + 1) * 64,
                                            bi * 128:(bi + 1) * 128],
                                        lhsT=V_t[b + o][:, h * 64:(h + 1) * 64],
                                        rhs=Q[:, hh * 1024 + bi * 256 + o * 128:
                                              hh * 1024 + bi * 256 + (o + 1) * 128],
                                        start=(o == 0),
                                        stop=(o == 1),
                                    )
                        nc.vector.tensor_mul(
                            outT_t[hp][:, ch * 512:(ch + 1) * 512],
                            pav[:],
                            Iv_pair[:, hp * HALF + ch * 512:
                                    hp * HALF + (ch + 1) * 512],
                        )
                    for bi in range(CB):
                        b = ch * CB + bi
                        cols = slice(b * 128, (b + 1) * 128)
                        pf = psf.tile([128, D], F32, tag="pf")
                        for t in range(4):
                            nc.tensor.matmul(
                                pf[:],
                                lhsT=outT_t[t][:, cols],
                                rhs=wob_t[:, t * D:(t + 1) * D],
                                start=(t == 0),
                                stop=(t == 3),
                            )
                        fin = fpool.tile([128, D], BF16, tag="fin")
                        nc.vector.tensor_add(fin[:], pf[:], bout_t[:])
                        eng = nc.sync if b % 2 == 0 else nc.scalar
                        eng.dma_start(out[cols, :], fin[:])

    nc.compile()
    return nc


def _make_in_maps(x, W_v, W_sigma, b_sigma, W_out, b_out):
    bf = ml_dtypes.bfloat16
    m2r1 = np.empty((128, 256), dtype=np.float32)
    p = np.arange(128, dtype=np.float32)[:, None]
    q = np.arange(128, dtype=np.float32)[None, :]
    for o in range(2):
        m2r1[:, o * 128:(o + 1) * 128] = np.abs(q - p + 64.0 - 128.0 * o)
    identb = np.eye(128, dtype=np.float32)
    cpak_bf = np.concatenate([m2r1, identb], axis=1).astype(bf)

    wvb4 = np.concatenate([W_v.astype(bf)[i * 128:(i + 1) * 128]
                           for i in range(4)], axis=1)
    wob4 = np.concatenate([W_out.astype(bf)[i * 128:(i + 1) * 128]
                           for i in range(4)], axis=1)
    wsb4 = np.concatenate([W_sigma.astype(bf)[i * 128:(i + 1) * 128]
                           for i in range(4)], axis=1)
    bsig_b = np.broadcast_to(b_sigma[None, :], (128, H)).astype(np.float32)
    bout_b = np.broadcast_to(b_out[None, :], (128, D)).copy().astype(np.float32)

    in_maps = []
    for c in range(8):
        bb, half = c // 2, c % 2
        i_start = half * HALF
        # j rows [i_start - W, i_start + HALF + W), zero-padded at seq ends
        xp = np.zeros((NJ, D), dtype=np.float32)
        j_lo = max(0, i_start - W)
        j_hi = min(N, i_start + HALF + W)
        xp[j_lo - (i_start - W):j_hi - (i_start - W)] = x[bb, j_lo:j_hi]
        xT = np.ascontiguousarray(xp.T.astype(bf))     # [512, 1152]

        pcol = np.arange(128, dtype=np.float32)[:, None]
        blk = np.arange(NBLK, dtype=np.float32)[None, :]
        i_abs = i_start + blk * 128 + pcol
        ivp1 = np.repeat(-(i_abs + 1.0), H, axis=1).astype(np.float32)
        ivnm = np.repeat(-(float(N) - i_abs), H, axis=1).astype(np.float32)
        cpak_f32 = np.concatenate([bsig_b, ivp1, ivnm], axis=1)

        in_maps.append(
            {
                "xt0": np.ascontiguousarray(xT[0:128]),
                "xt1": np.ascontiguousarray(xT[128:256]),
                "xt2": np.ascontiguousarray(xT[256:384]),
                "xt3": np.ascontiguousarray(xT[384:512]),
                "wvb4": np.ascontiguousarray(wvb4),
                "wob4": np.ascontiguousarray(wob4),
                "wsb4": np.ascontiguousarray(wsb4),
                "cpak_bf": np.ascontiguousarray(cpak_bf),
                "cpak_f32": np.ascontiguousarray(cpak_f32),
                "bout": bout_b,
            }
        )
    return in_maps


def kernel(x, W_v, W_sigma, b_sigma, W_out, b_out):
    global _nc_cache
    x = np.asarray(x, dtype=np.float32)
    W_v = np.asarray(W_v, dtype=np.float32)
    W_sigma = np.asarray(W_sigma, dtype=np.float32)
    b_sigma = np.asarray(b_sigma, dtype=np.float32)
    W_out = np.asarray(W_out, dtype=np.float32)
    b_out = np.asarray(b_out, dtype=np.float32)

    if _nc_cache is None:
        _nc_cache = _build_nc()
    nc = _nc_cache

    in_maps = _make_in_maps(x, W_v, W_sigma, b_sigma, W_out, b_out)
    res = run_bass_kernel_spmd(nc, in_maps, core_ids=list(range(8)))

    out = np.empty((B, N, D), dtype=np.float32)
    for c in range(8):
        bb, half = c // 2, c % 2
        out[bb, half * HALF:(half + 1) * HALF, :] = \
            res.results[c]["out"].astype(np.float32)
    return out


# revision 15
# speedup vs baseline: 1.0329x; 1.0329x over previous
"""Distributed Trainium2 kernel for nn_Attention_2654289789382 (sparse_attention).

Math (reference):
    sigma = sigmoid(x @ W_sigma + b_sigma)           (b, h, n)
    den_i = exp(sigma)+1 ;  r_i = 1/den_i = sigmoid(-sigma)   in (0.2689, 0.5)
    prior[i,j] = softmax_j(-|i-j| * r_i)
    out = (prior @ v) reshaped @ W_out + b_out,  v = x @ W_v

Structure exploited:
  * r_i >= 0.2689  =>  banded attention, half-width 64: per 128-row i-block
    only 2 j-tiles of 128 (at +-64 offsets) contribute. x context trimmed
    to exactly the used 1152 rows per core.
  * softmax denominator in closed form (two-sided geometric series).
  * No Sigmoid table: sigma and r = 1/den via Exp + DVE reciprocal only;
    Exp table preloaded at t=0 by a dummy activation.
  * HAM warmup matmuls at t=0 so real GEMMs run at 2.4 GHz.
  * Loads striped over all 3 DMA issuers (sync/scalar HWDGE + gpsimd
    SWDGE, ~100 GB/s each), sigma-critical pieces first.
  * Tensor-queue order: warmup, sigma GEMM, V GEMM, then the tiny
    transposes (they wait on the sigma chain - placing them before V
    head-of-line-blocks the PE for ~7us).
  * -r / 1/den staged to DRAM h-major; R_all broadcast = per-head
    partition-striped stride-0 DMAs across all 3 issuers, interleaved
    with Iv pieces in consumption order.
  * Q = exp(|i-j| * -r): one bf16 DVE mult per (ch,hp,hh), ONE
    [128,2048] ScalarE Exp per (ch,hp). AV bf16, 2 heads / psum tile,
    4 blocks share a [128,512] psum so normalization is one DVE op per
    (ch,hp). out^T lands in proj lhsT layout; proj+bias+store per block.
  * Output stored bf16 (halves store traffic; ~1e-3 extra rel err).

Sharding: 8 cores = 4 batches x 2 sequence halves; no collectives.
"""

import numpy as np
import ml_dtypes

import concourse.bass as bass
import concourse.mybir as mybir
import concourse.tile as tile
from concourse import bacc
from concourse.bass_utils import run_bass_kernel_spmd

F32 = mybir.dt.float32
BF16 = mybir.dt.bfloat16

B, N, D = 4, 2048, 512
H, DH = 8, 64
HALF = N // 2            # 1024 rows per core
W = 64                   # band half-width
NJ = HALF + 2 * W        # 1152 used j rows per core
NBLK = HALF // 128       # 8 i-blocks per core
NVT = 9                  # V tiles: rows 64k..64k+128 within the 1152
CB = 4                   # i-blocks per chunk
NCH = NBLK // CB         # chunks

_nc_cache = None


def _build_nc():
    nc = bacc.Bacc("TRN2", target_bir_lowering=False, debug=False)

    xt0 = nc.dram_tensor("xt0", [128, NJ], BF16, kind="ExternalInput")
    xt1 = nc.dram_tensor("xt1", [128, NJ], BF16, kind="ExternalInput")
    xt2 = nc.dram_tensor("xt2", [128, NJ], BF16, kind="ExternalInput")
    xt3 = nc.dram_tensor("xt3", [128, NJ], BF16, kind="ExternalInput")
    wvb4 = nc.dram_tensor("wvb4", [128, 4 * D], BF16, kind="ExternalInput")
    wob4 = nc.dram_tensor("wob4", [128, 4 * D], BF16, kind="ExternalInput")
    wsb4 = nc.dram_tensor("wsb4", [128, 4 * H], BF16, kind="ExternalInput")
    cpak_bf = nc.dram_tensor("cpak_bf", [128, 384], BF16, kind="ExternalInput")
    cpak_f32 = nc.dram_tensor("cpak_f32", [128, 136], F32, kind="ExternalInput")
    bout = nc.dram_tensor("bout", [128, D], F32, kind="ExternalInput")
    out = nc.dram_tensor("out", [HALF, D], BF16, kind="ExternalOutput")
    negr_d = nc.dram_tensor("negr_d", [64, 128], BF16)
    inv_d = nc.dram_tensor("inv_d", [64, 128], BF16)

    EXP = mybir.ActivationFunctionType.Exp
    MUL = mybir.AluOpType.mult
    ADD = mybir.AluOpType.add

    with tile.TileContext(nc) as tc:
        with (
            tc.tile_pool(name="const", bufs=1) as cpool,
            tc.tile_pool(name="vpool", bufs=1) as vpool,
            tc.tile_pool(name="otpool", bufs=1) as otpool,
            tc.tile_pool(name="sg", bufs=1) as sgpool,
            tc.tile_pool(name="bc", bufs=1) as bcpool,
            tc.tile_pool(name="warm", bufs=1) as wpool,
        ):
            # ---- t=0: exp-table preload + HAM warmup (no data deps) ----
            dum = wpool.tile([128, 128], BF16, tag="dum")
            nc.vector.memset(dum[:], 0.25)
            dume = wpool.tile([128, 8], F32, tag="dume")
            nc.scalar.activation(dume[:], dum[:, 0:8], EXP)
            with tc.tile_pool(name="pswarm", bufs=1, space="PSUM") as psw:
                pw = psw.tile([128, 128], F32, tag="pw")
                for _ in range(40):
                    nc.tensor.matmul(pw[:], lhsT=dum[:], rhs=dum[:],
                                     start=True, stop=True)

            # ------------- loads: 3 issuers, critical-first -------------
            cf32_t = cpool.tile([128, 136], F32, tag="cf32")
            nc.sync.dma_start(cf32_t[:], cpak_f32[:, :])
            cbf_t = cpool.tile([128, 384], BF16, tag="cbf")
            nc.scalar.dma_start(cbf_t[:], cpak_bf[:, :])
            wsb_t = cpool.tile([128, 4 * H], BF16, tag="wsb")
            nc.gpsimd.dma_start(wsb_t[:], wsb4[:, :])

            xt_t = []
            for d, (dram, eng) in enumerate(
                [(xt0, nc.sync), (xt1, nc.scalar), (xt2, nc.gpsimd)]
            ):
                t = cpool.tile([128, NJ], BF16, tag=f"xt{d}")
                eng.dma_start(t[:], dram[:, :])
                xt_t.append(t)
            t3 = cpool.tile([128, NJ], BF16, tag="xt3")
            nc.sync.dma_start(t3[:, 0:576], xt3[:, 0:576])
            nc.scalar.dma_start(t3[:, 576:NJ], xt3[:, 576:NJ])
            xt_t.append(t3)

            wvb_t = cpool.tile([128, 4 * D], BF16, tag="wvb")
            nc.sync.dma_start(wvb_t[:, 0:D], wvb4[:, 0:D])
            nc.scalar.dma_start(wvb_t[:, D:2 * D], wvb4[:, D:2 * D])
            nc.gpsimd.dma_start(wvb_t[:, 2 * D:4 * D], wvb4[:, 2 * D:4 * D])
            wob_t = cpool.tile([128, 4 * D], BF16, tag="wob")
            bout_t = cpool.tile([128, D], F32, tag="bout")

            m2r_t = cbf_t[:, 0:256]
            identb = cbf_t[:, 256:384]
            bsig = cf32_t[:, 0:8]
            ivp1 = cf32_t[:, 8:72]
            ivnm = cf32_t[:, 72:136]

            # ------------- sigma GEMM (blocks at cols 64 + 128b) --------
            with tc.tile_pool(name="pss", bufs=1, space="PSUM") as pss:
                ps = pss.tile([128, NBLK * H], F32, tag="ps")
                for b in range(NBLK):
                    for dt in range(4):
                        nc.tensor.matmul(
                            ps[:, b * H:(b + 1) * H],
                            lhsT=xt_t[dt][:, 64 + b * 128:64 + (b + 1) * 128],
                            rhs=wsb_t[:, dt * H:(dt + 1) * H],
                            start=(dt == 0),
                            stop=(dt == 3),
                        )
                s_all = sgpool.tile([128, NBLK * H], F32, tag="s_all")
                nc.vector.tensor_tensor(
                    s_all[:].rearrange("p (b h) -> p b h", h=H),
                    ps[:].rearrange("p (b h) -> p b h", h=H),
                    bsig.rearrange("p (one h) -> p one h", one=1)
                    .broadcast_to((128, NBLK, H)),
                    op=ADD,
                )

            # ---- V GEMM + sigma chain, interleaved so no engine queue
            #      head-of-line-blocks another:
            #      PE queue: sigma MMs | V0-2 MMs | ptn,pti | V3-8 MMs | AV
            #      Scalar:   chain exps + stages BEFORE its V copies
            #      DVE:      chain arith, then V copies interleaved
            V_t = [vpool.tile([128, D], BF16, tag=f"V{k}", name=f"vt{k}")
                   for k in range(NVT)]
            psv = tc.alloc_tile_pool(name="psv", bufs=3, space="PSUM")
            pst = tc.alloc_tile_pool(name="pst", bufs=1, space="PSUM")
            pv_t = {}

            def v_mms(k):
                pv = psv.tile([128, D], F32, tag="pv", name=f"pv{k}")
                for dt in range(4):
                    nc.tensor.matmul(
                        pv[:],
                        lhsT=xt_t[dt][:, 128 * k:128 * k + 128],
                        rhs=wvb_t[:, dt * D:(dt + 1) * D],
                        start=(dt == 0),
                        stop=(dt == 3),
                    )
                pv_t[k] = pv

            def v_copy(k, eng):
                if eng == "s":
                    nc.scalar.copy(V_t[k][:], pv_t[k][:])
                else:
                    nc.vector.tensor_copy(V_t[k][:], pv_t[k][:])

            for k in range(3):
                v_mms(k)

            # ---- sigma chain (no Sigmoid tables) ----
            ems = sgpool.tile([128, NBLK * H], F32, tag="ems")
            nc.scalar.activation(ems[:], s_all[:], EXP, scale=-1.0)
            d1 = sgpool.tile([128, NBLK * H], F32, tag="d1")
            nc.vector.tensor_scalar(d1[:], ems[:], 1.0, None, ADD)
            sig = sgpool.tile([128, NBLK * H], F32, tag="sig")
            nc.vector.reciprocal(sig[:], d1[:])
            esg = sgpool.tile([128, NBLK * H], F32, tag="esg")
            nc.scalar.activation(esg[:], sig[:], EXP)
            den = sgpool.tile([128, NBLK * H], F32, tag="den")
            nc.vector.tensor_scalar(den[:], esg[:], 1.0, None, ADD)
            r_all = sgpool.tile([128, NBLK * H], F32, tag="r_all")
            nc.vector.reciprocal(r_all[:], den[:])

            negr_b = sgpool.tile([128, NBLK * H], BF16, tag="negr_b")
            nc.vector.tensor_scalar(
                negr_b[:].rearrange("p (h b) -> p h b", b=NBLK),
                r_all[:].rearrange("p (b h) -> p h b", h=H),
                -1.0, None, MUL,
            )
            ptn = pst.tile([64, 128], BF16, tag="ptn")
            nc.tensor.transpose(ptn[:], negr_b[:], identb)
            negrT = sgpool.tile([64, 128], BF16, tag="negrT")
            nc.scalar.copy(negrT[:], ptn[:])
            nc.sync.dma_start(negr_d.ap(), negrT[:, :])

            # ---- 1/den closed form ----
            z = sgpool.tile([128, NBLK * H], F32, tag="z")
            nc.scalar.activation(z[:], r_all[:], EXP, scale=-1.0)
            argA = sgpool.tile([128, NBLK * H], F32, tag="argA")
            nc.vector.tensor_mul(argA[:], r_all[:], ivp1)
            expA = sgpool.tile([128, NBLK * H], F32, tag="expA")
            nc.scalar.activation(expA[:], argA[:], EXP)
            argB = sgpool.tile([128, NBLK * H], F32, tag="argB")
            nc.vector.tensor_mul(argB[:], r_all[:], ivnm)
            expB = sgpool.tile([128, NBLK * H], F32, tag="expB")
            nc.scalar.activation(expB[:], argB[:], EXP)
            w = sgpool.tile([128, NBLK * H], F32, tag="w")
            nc.vector.tensor_scalar(w[:], z[:], -1.0, 1.0, MUL, ADD)
            t1 = sgpool.tile([128, NBLK * H], F32, tag="t1")
            nc.vector.tensor_scalar_mul(t1[:], z[:], 2.0)
            nc.vector.tensor_sub(t1[:], t1[:], expA[:])
            nc.vector.tensor_sub(t1[:], t1[:], expB[:])
            u = sgpool.tile([128, NBLK * H], F32, tag="u")
            nc.vector.tensor_add(u[:], w[:], t1[:])
            ru = sgpool.tile([128, NBLK * H], F32, tag="ru")
            nc.vector.reciprocal(ru[:], u[:])
            inv_c = sgpool.tile([128, NBLK * H], F32, tag="inv_c")
            nc.vector.tensor_mul(inv_c[:], w[:], ru[:])
            inv_b = sgpool.tile([128, NBLK * H], BF16, tag="inv_b")
            nc.vector.tensor_copy(
                inv_b[:].rearrange("p (h b) -> p h b", b=NBLK),
                inv_c[:].rearrange("p (b h) -> p h b", h=H),
            )
            pti = pst.tile([64, 128], BF16, tag="pti")
            nc.tensor.transpose(pti[:], inv_b[:], identb)
            invT = sgpool.tile([64, 128], BF16, tag="invT")
            nc.scalar.copy(invT[:], pti[:])
            nc.scalar.dma_start(inv_d.ap(), invT[:, :])

            # ---- broadcasts, split per (piece, chunk), in consumption
            #      order, round-robined across the 3 DMA issuers.  wob /
            #      bout inserted after the ch0 pieces (needed ~first proj).
            R_all = bcpool.tile([128, H * HALF], BF16, tag="R_all")
            Iv_pair = bcpool.tile([128, 4 * HALF], BF16, tag="Iv_pair")
            ISS = [nc.sync, nc.scalar, nc.gpsimd]
            nsrc = negr_d.ap().rearrange("r p -> (r p)").unsqueeze(0)
            isrc = inv_d.ap().rearrange("r p -> (r p)").unsqueeze(0)

            def jobs_for_chunk(ch):
                c0 = ch * 512
                jobs = []
                for hp in range(4):
                    for hh in range(2):
                        h = 2 * hp + hh
                        for p0, p1 in ((0, 64), (64, 128)):
                            jobs.append((
                                R_all[p0:p1, h * HALF + c0:h * HALF + c0 + 512],
                                nsrc[:, h * HALF + c0:h * HALF + c0 + 512]
                                .to_broadcast((64, 512)),
                            ))
                    for half in range(2):
                        h = 2 * hp + half
                        jobs.append((
                            Iv_pair[half * 64:(half + 1) * 64,
                                    hp * HALF + c0:hp * HALF + c0 + 512],
                            isrc[:, h * HALF + c0:h * HALF + c0 + 512]
                            .to_broadcast((64, 512)),
                        ))
                return jobs

            ji = 0
            for dst, src in jobs_for_chunk(0):
                ISS[ji % 3].dma_start(dst, src)
                ji += 1
            # wob / bout land between ch0 and ch1 broadcast waves
            nc.sync.dma_start(wob_t[:, 0:2 * D], wob4[:, 0:2 * D])
            nc.scalar.dma_start(wob_t[:, 2 * D:4 * D], wob4[:, 2 * D:4 * D])
            nc.gpsimd.dma_start(bout_t[:], bout[:, :])
            for dst, src in jobs_for_chunk(1):
                ISS[ji % 3].dma_start(dst, src)
                ji += 1

            # ---- rest of V GEMM + all psum evacuations ----
            for k in range(3, NVT):
                v_copy(k - 3, "s" if (k - 3) % 2 == 0 else "v")
                v_mms(k)
            for k in range(NVT - 3, NVT):
                v_copy(k, "s" if k % 2 == 0 else "v")
            pst.release()
            psv.release()

            outT_t = []
            for t in range(4):
                oT = otpool.tile([128, HALF], BF16, tag=f"oT{t}")
                outT_t.append(oT)

            # ---------------- main loop ----------------
            with (
                tc.tile_pool(name="qp", bufs=4) as qpool,
                tc.tile_pool(name="fin", bufs=3) as fpool,
                tc.tile_pool(name="psa", bufs=3, space="PSUM") as psa,
                tc.tile_pool(name="psf", bufs=2, space="PSUM") as psf,
            ):
                for ch in range(NCH):
                    for hp in range(4):
                        Q = qpool.tile([128, 2 * CB * 256], BF16, tag="Q")
                        ARG = qpool.tile([128, 2 * CB * 256], BF16, tag="ARG")
                        for hh in range(2):
                            h = 2 * hp + hh
                            R = R_all[:, h * HALF + ch * CB * 128:
                                      h * HALF + (ch + 1) * CB * 128]
                            nc.vector.tensor_tensor(
                                ARG[:, hh * 1024:(hh + 1) * 1024]
                                .rearrange("p (b o q) -> p b o q", b=CB, o=2),
                                m2r_t
                                .rearrange("p (one o q) -> p one o q", one=1, o=2)
                                .broadcast_to((128, CB, 2, 128)),
                                R.rearrange("p (b one q) -> p b one q", b=CB, one=1)
                                .broadcast_to((128, CB, 2, 128)),
                                op=MUL,
                            )
                        nc.scalar.activation(Q[:], ARG[:], EXP)
                        pav = psa.tile([128, CB * 128], F32, tag="pav")
                        for bi in range(CB):
                            b = ch * CB + bi
                            for hh in range(2):
                                h = 2 * hp + hh
                                for o in range(2):
                                    nc.tensor.matmul(
                                        pav[hh * 64:(hh + 1) * 64,
                                            bi * 128:(bi + 1) * 128],
                                        lhsT=V_t[b + o][:, h * 64:(h + 1) * 64],
                                        rhs=Q[:, hh * 1024 + bi * 256 + o * 128:
                                              hh * 1024 + bi * 256 + (o + 1) * 128],
                                        start=(o == 0),
                                        stop=(o == 1),
                                    )
                        nc.vector.tensor_mul(
                            outT_t[hp][:, ch * 512:(ch + 1) * 512],
                            pav[:],
                            Iv_pair[:, hp * HALF + ch * 512:
                                    hp * HALF + (ch + 1) * 512],
                        )
                    for bi in range(CB):
                        b = ch * CB + bi
                        cols = slice(b * 128, (b + 1) * 128)
                        pf = psf.tile([128, D], F32, tag="pf")
                        for t in range(4):
                            nc.tensor.matmul(
                                pf[:],
                                lhsT=outT_t[t][:, cols],
                                rhs=wob_t[:, t * D:(t + 1) * D],
                                start=(t == 0),
                                stop=(t == 3),
                            )
                        fin = fpool.tile([128, D], BF16, tag="fin")
                        nc.vector.tensor_add(fin[:], pf[:], bout_t[:])
                        eng = nc.sync if b % 2 == 0 else nc.scalar
                        eng.dma_start(out[cols, :], fin[:])

    nc.compile()
    return nc


def _make_in_maps(x, W_v, W_sigma, b_sigma, W_out, b_out):
    bf = ml_dtypes.bfloat16
    m2r1 = np.empty((128, 256), dtype=np.float32)
    p = np.arange(128, dtype=np.float32)[:, None]
    q = np.arange(128, dtype=np.float32)[None, :]
    for o in range(2):
        m2r1[:, o * 128:(o + 1) * 128] = np.abs(q - p + 64.0 - 128.0 * o)
    identb = np.eye(128, dtype=np.float32)
    cpak_bf = np.concatenate([m2r1, identb], axis=1).astype(bf)

    wvb4 = np.concatenate([W_v.astype(bf)[i * 128:(i + 1) * 128]
                           for i in range(4)], axis=1)
    wob4 = np.concatenate([W_out.astype(bf)[i * 128:(i + 1) * 128]
                           for i in range(4)], axis=1)
    wsb4 = np.concatenate([W_sigma.astype(bf)[i * 128:(i + 1) * 128]
                           for i in range(4)], axis=1)
    bsig_b = np.broadcast_to(b_sigma[None, :], (128, H)).astype(np.float32)
    bout_b = np.broadcast_to(b_out[None, :], (128, D)).copy().astype(np.float32)

    in_maps = []
    for c in range(8):
        bb, half = c // 2, c % 2
        i_start = half * HALF
        # j rows [i_start - W, i_start + HALF + W), zero-padded at seq ends
        xp = np.zeros((NJ, D), dtype=np.float32)
        j_lo = max(0, i_start - W)
        j_hi = min(N, i_start + HALF + W)
        xp[j_lo - (i_start - W):j_hi - (i_start - W)] = x[bb, j_lo:j_hi]
        xT = np.ascontiguousarray(xp.T.astype(bf))     # [512, 1152]

        pcol = np.arange(128, dtype=np.float32)[:, None]
        blk = np.arange(NBLK, dtype=np.float32)[None, :]
        i_abs = i_start + blk * 128 + pcol
        ivp1 = np.repeat(-(i_abs + 1.0), H, axis=1).astype(np.float32)
        ivnm = np.repeat(-(float(N) - i_abs), H, axis=1).astype(np.float32)
        cpak_f32 = np.concatenate([bsig_b, ivp1, ivnm], axis=1)

        in_maps.append(
            {
                "xt0": np.ascontiguousarray(xT[0:128]),
                "xt1": np.ascontiguousarray(xT[128:256]),
                "xt2": np.ascontiguousarray(xT[256:384]),
                "xt3": np.ascontiguousarray(xT[384:512]),
                "wvb4": np.ascontiguousarray(wvb4),
                "wob4": np.ascontiguousarray(wob4),
                "wsb4": np.ascontiguousarray(wsb4),
                "cpak_bf": np.ascontiguousarray(cpak_bf),
                "cpak_f32": np.ascontiguousarray(cpak_f32),
                "bout": bout_b,
            }
        )
    return in_maps


def kernel(x, W_v, W_sigma, b_sigma, W_out, b_out):
    global _nc_cache
    x = np.asarray(x, dtype=np.float32)
    W_v = np.asarray(W_v, dtype=np.float32)
    W_sigma = np.asarray(W_sigma, dtype=np.float32)
    b_sigma = np.asarray(b_sigma, dtype=np.float32)
    W_out = np.asarray(W_out, dtype=np.float32)
    b_out = np.asarray(b_out, dtype=np.float32)

    if _nc_cache is None:
        _nc_cache = _build_nc()
    nc = _nc_cache

    in_maps = _make_in_maps(x, W_v, W_sigma, b_sigma, W_out, b_out)
    res = run_bass_kernel_spmd(nc, in_maps, core_ids=list(range(8)))

    out = np.empty((B, N, D), dtype=np.float32)
    for c in range(8):
        bb, half = c // 2, c % 2
        out[bb, half * HALF:(half + 1) * HALF, :] = \
            res.results[c]["out"].astype(np.float32)
    return out


# revision 18
# speedup vs baseline: 1.1405x; 1.1042x over previous
"""Distributed Trainium2 kernel for nn_Attention_2654289789382 (sparse_attention).

Math (reference):
    sigma = sigmoid(x @ W_sigma + b_sigma)           (b, h, n)
    den_i = exp(sigma)+1 ;  r_i = 1/den_i = sigmoid(-sigma)   in (0.2689, 0.5)
    prior[i,j] = softmax_j(-|i-j| * r_i)
    out = (prior @ v) reshaped @ W_out + b_out,  v = x @ W_v

Structure exploited:
  * r_i >= 0.2689  =>  banded attention, half-width 64: per 128-row i-block
    only 2 j-tiles of 128 (at +-64 offsets) contribute. x context trimmed
    to exactly the used 1152 rows per core.
  * softmax denominator in closed form (two-sided geometric series).
  * No Sigmoid table: sigma and r = 1/den via Exp + DVE reciprocal only;
    Exp table preloaded at t=0 by a dummy activation.
  * HAM warmup matmuls at t=0 so real GEMMs run at 2.4 GHz.
  * Loads striped over all 3 DMA issuers (sync/scalar HWDGE + gpsimd
    SWDGE, ~100 GB/s each), sigma-critical pieces first.
  * Tensor-queue order: warmup, sigma GEMM, V GEMM, then the tiny
    transposes (they wait on the sigma chain - placing them before V
    head-of-line-blocks the PE for ~7us).
  * -r / 1/den staged to DRAM h-major; R_all broadcast = per-head
    partition-striped stride-0 DMAs across all 3 issuers, interleaved
    with Iv pieces in consumption order.
  * Q = exp(|i-j| * -r): one bf16 DVE mult per (ch,hp,hh), ONE
    [128,2048] ScalarE Exp per (ch,hp). AV bf16, 2 heads / psum tile,
    4 blocks share a [128,512] psum so normalization is one DVE op per
    (ch,hp). out^T lands in proj lhsT layout; proj+bias+store per block.
  * Output stored bf16 (halves store traffic; ~1e-3 extra rel err).

Sharding: 8 cores = 4 batches x 2 sequence halves; no collectives.
"""

import numpy as np
import ml_dtypes

import concourse.bass as bass
import concourse.mybir as mybir
import concourse.tile as tile
from concourse import bacc
from concourse.bass_utils import run_bass_kernel_spmd

F32 = mybir.dt.float32
BF16 = mybir.dt.bfloat16

B, N, D = 4, 2048, 512
H, DH = 8, 64
HALF = N // 2            # 1024 rows per core
W = 64                   # band half-width
NJ = HALF + 2 * W        # 1152 used j rows per core
NBLK = HALF // 128       # 8 i-blocks per core
NVT = 9                  # V tiles: rows 64k..64k+128 within the 1152
CB = 4                   # i-blocks per chunk
NCH = NBLK // CB         # chunks

_nc_cache = None


def _build_nc():
    nc = bacc.Bacc("TRN2", target_bir_lowering=False, debug=False)

    xt0 = nc.dram_tensor("xt0", [128, NJ], BF16, kind="ExternalInput")
    xt1 = nc.dram_tensor("xt1", [128, NJ], BF16, kind="ExternalInput")
    xt2 = nc.dram_tensor("xt2", [128, NJ], BF16, kind="ExternalInput")
    xt3 = nc.dram_tensor("xt3", [128, NJ], BF16, kind="ExternalInput")
    wvb4 = nc.dram_tensor("wvb4", [128, 4 * D], BF16, kind="ExternalInput")
    wob4 = nc.dram_tensor("wob4", [128, 4 * D], BF16, kind="ExternalInput")
    wsb4 = nc.dram_tensor("wsb4", [128, 4 * H], BF16, kind="ExternalInput")
    cpak_bf = nc.dram_tensor("cpak_bf", [128, 384], BF16, kind="ExternalInput")
    cpak_f32 = nc.dram_tensor("cpak_f32", [128, 136], F32, kind="ExternalInput")
    bout = nc.dram_tensor("bout", [128, D], BF16, kind="ExternalInput")
    out = nc.dram_tensor("out", [HALF, D], BF16, kind="ExternalOutput")
    negr_d = nc.dram_tensor("negr_d", [64, 128], BF16)
    inv_d = nc.dram_tensor("inv_d", [64, 128], BF16)

    EXP = mybir.ActivationFunctionType.Exp
    MUL = mybir.AluOpType.mult
    ADD = mybir.AluOpType.add

    with tile.TileContext(nc) as tc:
        with (
            tc.tile_pool(name="const", bufs=1) as cpool,
            tc.tile_pool(name="vpool", bufs=1) as vpool,
            tc.tile_pool(name="otpool", bufs=1) as otpool,
            tc.tile_pool(name="sg", bufs=1) as sgpool,
            tc.tile_pool(name="bc", bufs=1) as bcpool,
            tc.tile_pool(name="warm", bufs=1) as wpool,
        ):
            # ---- t=0: exp-table preload + HAM warmup (no data deps) ----
            dum = wpool.tile([128, 128], BF16, tag="dum")
            nc.vector.memset(dum[:], 0.25)
            dume = wpool.tile([128, 8], F32, tag="dume")
            nc.scalar.activation(dume[:], dum[:, 0:8], EXP)
            with tc.tile_pool(name="pswarm", bufs=1, space="PSUM") as psw:
                pw = psw.tile([128, 128], F32, tag="pw")
                for _ in range(40):
                    nc.tensor.matmul(pw[:], lhsT=dum[:], rhs=dum[:],
                                     start=True, stop=True)

            # ------------- loads: 3 issuers, critical-first -------------
            cf32_t = cpool.tile([128, 136], F32, tag="cf32")
            nc.sync.dma_start(cf32_t[:], cpak_f32[:, :])
            cbf_t = cpool.tile([128, 384], BF16, tag="cbf")
            nc.scalar.dma_start(cbf_t[:], cpak_bf[:, :])
            wsb_t = cpool.tile([128, 4 * H], BF16, tag="wsb")
            nc.gpsimd.dma_start(wsb_t[:], wsb4[:, :])

            xt_t = []
            for d, (dram, eng) in enumerate(
                [(xt0, nc.sync), (xt1, nc.scalar), (xt2, nc.gpsimd)]
            ):
                t = cpool.tile([128, NJ], BF16, tag=f"xt{d}")
                eng.dma_start(t[:], dram[:, :])
                xt_t.append(t)
            t3 = cpool.tile([128, NJ], BF16, tag="xt3")
            nc.sync.dma_start(t3[:, 0:576], xt3[:, 0:576])
            nc.scalar.dma_start(t3[:, 576:NJ], xt3[:, 576:NJ])
            xt_t.append(t3)

            wvb_t = cpool.tile([128, 4 * D], BF16, tag="wvb")
            nc.sync.dma_start(wvb_t[:, 0:D], wvb4[:, 0:D])
            nc.scalar.dma_start(wvb_t[:, D:2 * D], wvb4[:, D:2 * D])
            nc.gpsimd.dma_start(wvb_t[:, 2 * D:4 * D], wvb4[:, 2 * D:4 * D])
            wob_t = cpool.tile([128, 4 * D], BF16, tag="wob")
            bout_t = cpool.tile([128, D], BF16, tag="bout")

            m2r_t = cbf_t[:, 0:256]
            identb = cbf_t[:, 256:384]
            bsig = cf32_t[:, 0:8]
            ivp1 = cf32_t[:, 8:72]
            ivnm = cf32_t[:, 72:136]

            # ------------- sigma GEMM (blocks at cols 64 + 128b) --------
            with tc.tile_pool(name="pss", bufs=1, space="PSUM") as pss:
                ps = pss.tile([128, NBLK * H], F32, tag="ps")
                for b in range(NBLK):
                    for dt in range(4):
                        nc.tensor.matmul(
                            ps[:, b * H:(b + 1) * H],
                            lhsT=xt_t[dt][:, 64 + b * 128:64 + (b + 1) * 128],
                            rhs=wsb_t[:, dt * H:(dt + 1) * H],
                            start=(dt == 0),
                            stop=(dt == 3),
                        )
                s_all = sgpool.tile([128, NBLK * H], F32, tag="s_all")
                nc.vector.tensor_tensor(
                    s_all[:].rearrange("p (b h) -> p b h", h=H),
                    ps[:].rearrange("p (b h) -> p b h", h=H),
                    bsig.rearrange("p (one h) -> p one h", one=1)
                    .broadcast_to((128, NBLK, H)),
                    op=ADD,
                )

            # ---- V GEMM + sigma chain, interleaved so no engine queue
            #      head-of-line-blocks another:
            #      PE queue: sigma MMs | V0-2 MMs | ptn,pti | V3-8 MMs | AV
            #      Scalar:   chain exps + stages BEFORE its V copies
            #      DVE:      chain arith, then V copies interleaved
            V_t = [vpool.tile([128, D], BF16, tag=f"V{k}", name=f"vt{k}")
                   for k in range(NVT)]
            psv = tc.alloc_tile_pool(name="psv", bufs=3, space="PSUM")
            pst = tc.alloc_tile_pool(name="pst", bufs=1, space="PSUM")
            pv_t = {}

            def v_mms(k):
                pv = psv.tile([128, D], F32, tag="pv", name=f"pv{k}")
                for dt in range(4):
                    nc.tensor.matmul(
                        pv[:],
                        lhsT=xt_t[dt][:, 128 * k:128 * k + 128],
                        rhs=wvb_t[:, dt * D:(dt + 1) * D],
                        start=(dt == 0),
                        stop=(dt == 3),
                    )
                pv_t[k] = pv

            def v_copy(k, eng):
                if eng == "s":
                    nc.scalar.copy(V_t[k][:], pv_t[k][:])
                else:
                    nc.vector.tensor_copy(V_t[k][:], pv_t[k][:])

            for k in range(3):
                v_mms(k)

            # ---- sigma chain (no Sigmoid tables) ----
            ems = sgpool.tile([128, NBLK * H], F32, tag="ems")
            nc.scalar.activation(ems[:], s_all[:], EXP, scale=-1.0)
            d1 = sgpool.tile([128, NBLK * H], F32, tag="d1")
            nc.vector.tensor_scalar(d1[:], ems[:], 1.0, None, ADD)
            sig = sgpool.tile([128, NBLK * H], F32, tag="sig")
            nc.vector.reciprocal(sig[:], d1[:])
            esg = sgpool.tile([128, NBLK * H], F32, tag="esg")
            nc.scalar.activation(esg[:], sig[:], EXP)
            den = sgpool.tile([128, NBLK * H], F32, tag="den")
            nc.vector.tensor_scalar(den[:], esg[:], 1.0, None, ADD)
            r_all = sgpool.tile([128, NBLK * H], F32, tag="r_all")
            nc.vector.reciprocal(r_all[:], den[:])

            negr_b = sgpool.tile([128, NBLK * H], BF16, tag="negr_b")
            nc.vector.tensor_scalar(
                negr_b[:].rearrange("p (h b) -> p h b", b=NBLK),
                r_all[:].rearrange("p (b h) -> p h b", h=H),
                -1.0, None, MUL,
            )
            ptn = pst.tile([64, 128], BF16, tag="ptn")
            nc.tensor.transpose(ptn[:], negr_b[:], identb)
            negrT = sgpool.tile([64, 128], BF16, tag="negrT")
            nc.scalar.copy(negrT[:], ptn[:])
            nc.sync.dma_start(negr_d.ap(), negrT[:, :])

            # ---- 1/den closed form ----
            z = sgpool.tile([128, NBLK * H], F32, tag="z")
            nc.scalar.activation(z[:], r_all[:], EXP, scale=-1.0)
            argA = sgpool.tile([128, NBLK * H], F32, tag="argA")
            nc.vector.tensor_mul(argA[:], r_all[:], ivp1)
            expA = sgpool.tile([128, NBLK * H], F32, tag="expA")
            nc.scalar.activation(expA[:], argA[:], EXP)
            argB = sgpool.tile([128, NBLK * H], F32, tag="argB")
            nc.vector.tensor_mul(argB[:], r_all[:], ivnm)
            expB = sgpool.tile([128, NBLK * H], F32, tag="expB")
            nc.scalar.activation(expB[:], argB[:], EXP)
            w = sgpool.tile([128, NBLK * H], F32, tag="w")
            nc.vector.tensor_scalar(w[:], z[:], -1.0, 1.0, MUL, ADD)
            t1 = sgpool.tile([128, NBLK * H], F32, tag="t1")
            nc.vector.tensor_scalar_mul(t1[:], z[:], 2.0)
            nc.vector.tensor_sub(t1[:], t1[:], expA[:])
            nc.vector.tensor_sub(t1[:], t1[:], expB[:])
            u = sgpool.tile([128, NBLK * H], F32, tag="u")
            nc.vector.tensor_add(u[:], w[:], t1[:])
            ru = sgpool.tile([128, NBLK * H], F32, tag="ru")
            nc.vector.reciprocal(ru[:], u[:])
            inv_c = sgpool.tile([128, NBLK * H], F32, tag="inv_c")
            nc.vector.tensor_mul(inv_c[:], w[:], ru[:])
            inv_b = sgpool.tile([128, NBLK * H], BF16, tag="inv_b")
            nc.vector.tensor_copy(
                inv_b[:].rearrange("p (h b) -> p h b", b=NBLK),
                inv_c[:].rearrange("p (b h) -> p h b", h=H),
            )
            pti = pst.tile([64, 128], BF16, tag="pti")
            nc.tensor.transpose(pti[:], inv_b[:], identb)
            invT = sgpool.tile([64, 128], BF16, tag="invT")
            nc.scalar.copy(invT[:], pti[:])
            nc.sync.dma_start(inv_d.ap(), invT[:, :])

            # ---- broadcasts: R whole-[128,512] per (head,chunk), Iv
            #      [64,512] pieces, consumption order, sync+gpsimd ONLY
            #      (scalar's FIFO must stay clear for Q exps).
            R_all = bcpool.tile([128, H * HALF], BF16, tag="R_all")
            Iv_pair = bcpool.tile([128, 4 * HALF], BF16, tag="Iv_pair")
            nsrc = negr_d.ap().rearrange("r p -> (r p)").unsqueeze(0)
            isrc = inv_d.ap().rearrange("r p -> (r p)").unsqueeze(0)

            def bcast_wave(ch):
                c0 = ch * 512
                for hp in range(4):
                    for hh in range(2):
                        h = 2 * hp + hh
                        eng = nc.gpsimd if hh == 0 else nc.sync
                        eng.dma_start(
                            R_all[:, h * HALF + c0:h * HALF + c0 + 512],
                            nsrc[:, h * HALF + c0:h * HALF + c0 + 512]
                            .to_broadcast((128, 512)),
                        )
                    for half in range(2):
                        h = 2 * hp + half
                        eng = nc.sync if half == 0 else nc.gpsimd
                        eng.dma_start(
                            Iv_pair[half * 64:(half + 1) * 64,
                                    hp * HALF + c0:hp * HALF + c0 + 512],
                            isrc[:, h * HALF + c0:h * HALF + c0 + 512]
                            .to_broadcast((64, 512)),
                        )

            bcast_wave(0)
            nc.sync.dma_start(wob_t[:, 0:2 * D], wob4[:, 0:2 * D])
            nc.gpsimd.dma_start(wob_t[:, 2 * D:4 * D], wob4[:, 2 * D:4 * D])
            nc.gpsimd.dma_start(bout_t[:], bout[:, :])
            bcast_wave(1)

            # ---- rest of V GEMM + all psum evacuations ----
            for k in range(3, NVT):
                v_copy(k - 3, "s" if (k - 3) in (0, 2) else "v")
                v_mms(k)
            for k in range(NVT - 3, NVT):
                v_copy(k, "v")
            pst.release()
            psv.release()

            outT_t = []
            for t in range(4):
                oT = otpool.tile([128, HALF], BF16, tag=f"oT{t}")
                outT_t.append(oT)

            # ---------------- main loop (Q-gen pipelined 1 ahead) ------
            with (
                tc.tile_pool(name="qp", bufs=4) as qpool,
                tc.tile_pool(name="fin", bufs=3) as fpool,
                tc.tile_pool(name="psa", bufs=3, space="PSUM") as psa,
                tc.tile_pool(name="psf", bufs=2, space="PSUM") as psf,
            ):
                def gen_q(ch, hp):
                    Q = qpool.tile([128, 2 * CB * 256], BF16, tag="Q",
                                   name=f"Q{ch}{hp}")
                    ARG = qpool.tile([128, 2 * CB * 256], BF16, tag="ARG",
                                     name=f"A{ch}{hp}")
                    for hh in range(2):
                        h = 2 * hp + hh
                        R = R_all[:, h * HALF + ch * CB * 128:
                                  h * HALF + (ch + 1) * CB * 128]
                        for o in range(2):
                            nc.vector.tensor_tensor(
                                ARG[:, hh * 1024:(hh + 1) * 1024]
                                .rearrange("p (b o q) -> p b o q", b=CB, o=2)
                                [:, :, o],
                                m2r_t[:, o * 128:(o + 1) * 128]
                                .rearrange("p (one q) -> p one q", one=1)
                                .broadcast_to((128, CB, 128)),
                                R.rearrange("p (b q) -> p b q", q=128),
                                op=MUL,
                            )
                    nc.scalar.activation(Q[:], ARG[:], EXP)
                    return Q

                iters = [(ch, hp) for ch in range(NCH) for hp in range(4)]
                Qcur = gen_q(0, 0)
                for it, (ch, hp) in enumerate(iters):
                    if it + 1 < len(iters):
                        Qnext = gen_q(*iters[it + 1])
                    pav = psa.tile([128, CB * 128], F32, tag="pav",
                                   name=f"pav{it}")
                    for bi in range(CB):
                        b = ch * CB + bi
                        for hh in range(2):
                            h = 2 * hp + hh
                            for o in range(2):
                                nc.tensor.matmul(
                                    pav[hh * 64:(hh + 1) * 64,
                                        bi * 128:(bi + 1) * 128],
                                    lhsT=V_t[b + o][:, h * 64:(h + 1) * 64],
                                    rhs=Qcur[:, hh * 1024 + bi * 256 + o * 128:
                                             hh * 1024 + bi * 256 + (o + 1) * 128],
                                    start=(o == 0),
                                    stop=(o == 1),
                                )
                    nc.vector.tensor_mul(
                        outT_t[hp][:, ch * 512:(ch + 1) * 512],
                        pav[:],
                        Iv_pair[:, hp * HALF + ch * 512:
                                hp * HALF + (ch + 1) * 512],
                    )
                    Qcur = Qnext
                    if hp == 3:
                        for bi in range(CB):
                            b = ch * CB + bi
                            cols = slice(b * 128, (b + 1) * 128)
                            pf = psf.tile([128, D], F32, tag="pf",
                                          name=f"pf{b}")
                            for t in range(4):
                                nc.tensor.matmul(
                                    pf[:],
                                    lhsT=outT_t[t][:, cols],
                                    rhs=wob_t[:, t * D:(t + 1) * D],
                                    start=(t == 0),
                                    stop=(t == 3),
                                )
                            fin = fpool.tile([128, D], BF16, tag="fin",
                                             name=f"fin{b}")
                            nc.vector.tensor_add(fin[:], pf[:], bout_t[:])
                            eng = nc.sync if b % 2 == 0 else nc.gpsimd
                            eng.dma_start(out[cols, :], fin[:])

    nc.compile()
    return nc


def _make_in_maps(x, W_v, W_sigma, b_sigma, W_out, b_out):
    bf = ml_dtypes.bfloat16
    m2r1 = np.empty((128, 256), dtype=np.float32)
    p = np.arange(128, dtype=np.float32)[:, None]
    q = np.arange(128, dtype=np.float32)[None, :]
    for o in range(2):
        m2r1[:, o * 128:(o + 1) * 128] = np.abs(q - p + 64.0 - 128.0 * o)
    identb = np.eye(128, dtype=np.float32)
    cpak_bf = np.concatenate([m2r1, identb], axis=1).astype(bf)

    wvb4 = np.concatenate([W_v.astype(bf)[i * 128:(i + 1) * 128]
                           for i in range(4)], axis=1)
    wob4 = np.concatenate([W_out.astype(bf)[i * 128:(i + 1) * 128]
                           for i in range(4)], axis=1)
    wsb4 = np.concatenate([W_sigma.astype(bf)[i * 128:(i + 1) * 128]
                           for i in range(4)], axis=1)
    bsig_b = np.broadcast_to(b_sigma[None, :], (128, H)).astype(np.float32)
    bout_b = np.broadcast_to(b_out[None, :], (128, D)).copy().astype(bf)

    in_maps = []
    for c in range(8):
        bb, half = c // 2, c % 2
        i_start = half * HALF
        # j rows [i_start - W, i_start + HALF + W), zero-padded at seq ends
        xp = np.zeros((NJ, D), dtype=np.float32)
        j_lo = max(0, i_start - W)
        j_hi = min(N, i_start + HALF + W)
        xp[j_lo - (i_start - W):j_hi - (i_start - W)] = x[bb, j_lo:j_hi]
        xT = np.ascontiguousarray(xp.T.astype(bf))     # [512, 1152]

        pcol = np.arange(128, dtype=np.float32)[:, None]
        blk = np.arange(NBLK, dtype=np.float32)[None, :]
        i_abs = i_start + blk * 128 + pcol
        ivp1 = np.repeat(-(i_abs + 1.0), H, axis=1).astype(np.float32)
        ivnm = np.repeat(-(float(N) - i_abs), H, axis=1).astype(np.float32)
        cpak_f32 = np.concatenate([bsig_b, ivp1, ivnm], axis=1)

        in_maps.append(
            {
                "xt0": np.ascontiguousarray(xT[0:128]),
                "xt1": np.ascontiguousarray(xT[128:256]),
                "xt2": np.ascontiguousarray(xT[256:384]),
                "xt3": np.ascontiguousarray(xT[384:512]),
                "wvb4": np.ascontiguousarray(wvb4),
                "wob4": np.ascontiguousarray(wob4),
                "wsb4": np.ascontiguousarray(wsb4),
                "cpak_bf": np.ascontiguousarray(cpak_bf),
                "cpak_f32": np.ascontiguousarray(cpak_f32),
                "bout": bout_b,
            }
        )
    return in_maps


def kernel(x, W_v, W_sigma, b_sigma, W_out, b_out):
    global _nc_cache
    x = np.asarray(x, dtype=np.float32)
    W_v = np.asarray(W_v, dtype=np.float32)
    W_sigma = np.asarray(W_sigma, dtype=np.float32)
    b_sigma = np.asarray(b_sigma, dtype=np.float32)
    W_out = np.asarray(W_out, dtype=np.float32)
    b_out = np.asarray(b_out, dtype=np.float32)

    if _nc_cache is None:
        _nc_cache = _build_nc()
    nc = _nc_cache

    in_maps = _make_in_maps(x, W_v, W_sigma, b_sigma, W_out, b_out)
    res = run_bass_kernel_spmd(nc, in_maps, core_ids=list(range(8)))

    out = np.empty((B, N, D), dtype=np.float32)
    for c in range(8):
        bb, half = c // 2, c % 2
        out[bb, half * HALF:(half + 1) * HALF, :] = \
            res.results[c]["out"].astype(np.float32)
    return out
